# revision 9
# baseline (speedup 1.0000x reference)
"""Bass/TRN2 kernel for nn_BMM_S8T_S8N_S8T:
    out[b,m,n] = sat_i8(round(alpha * sum_k a[b,m,k] * b[b,n,k]))
with a: (32, 2048, 64) int8, b: (32, 2048, 64) int8, alpha: f32 scalar.

Sharding: batch dim 32 -> 8 cores x 4 batches (pure data parallel, no
cross-core communication).

Per-core design notes:
  - int8 matmul is not supported by the PE; bf16 x bf16 -> f32 PSUM is exact
    for int8 operands (products < 2^16, sums of 64 < 2^24), so inputs are
    converted to bf16 (and pre-transposed to [K, seq] layout) on host.
  - The 4 local batches are stacked in pairs along SBUF partitions:
    partitions 0-63 hold batch 2p's K=64, partitions 64-127 batch 2p+1's.
    Row-tiled matmuls (row groups 0 and 2) run the two batches' K=64
    contractions on the 128x128 PE array.
  - Requant drain (PSUM f32 -> SBUF int8, scale + round-half-even + saturate)
    is the hard floor: only VectorE (0.96 GHz, 1 elem/lane/cyc from f32 PSUM)
    and ScalarE (1.2 GHz, 1 elem/lane/cyc) can read PSUM. Both engines drain
    in parallel on different PSUM banks, in [128, 1024] (2-bank) units so
    fills overlap drains within the 8 banks. A single tensor_scalar_mul /
    activation(Copy, scale) does the whole requant bit-exactly.
  - The 16 DMA engines are per-descriptor bound and serve descriptors in
    global trigger order, so all per-core input data is host-repacked into
    ONE dram tensor `w` laid out in exact streaming order and loaded as 5
    sequential pieces on the sync HWDGE; the first piece alone carries
    everything the first m-tile needs. Output DMAs ride the sync HWDGE too
    (a dedicated queue whose cross-engine waits stall nothing); the final
    m-tile's four [128,1024] halves are DMA'd individually from sync /
    gpsimd / scalar with the last two drains pinned to a known engine, so
    the exit tail is a single 128KB transfer.
"""

import numpy as np
import ml_dtypes

B, M, N, K = 32, 2048, 2048, 64
NCORES = 8
BPC = B // NCORES          # batches per core (4)
MT = M // 128              # m-tiles per batch (16)
NHALF = 2                  # two 1024-col drain units per m-tile row block
UNIT = N // NHALF          # 1024 columns per drain unit
WCOLS = 4 * M              # packed input tensor: 8192 bf16 columns

_CACHE = {}


def _build(alpha: float):
    import concourse.bacc as bacc
    import concourse.mybir as mybir
    from concourse.tile import TileContext

    bf16 = mybir.dt.bfloat16
    f32 = mybir.dt.float32
    i8 = mybir.dt.int8

    nc = bacc.Bacc("TRN2")
    w = nc.dram_tensor("w", (128, WCOLS), bf16, kind="ExternalInput")
    out = nc.dram_tensor("out", (BPC, M, N), i8, kind="ExternalOutput")

    # engine load balancing between the two drain engines (ns per [128,1024]
    # unit, hardware-measured)
    DVE_NS, ACT_NS = 1146.0, 1035.0
    WARMUP_MM = 4  # filler matmuls while the first input piece streams in

    # scratch operand for warm-up matmuls: deliberately never written (the
    # values don't matter and the scratch PSUM bank is never read); a raw
    # (non-pool) tensor so Tile's release tracking doesn't object.
    wz = nc.alloc_sbuf_tensor("wz", [128, 512], bf16)

    with TileContext(nc) as tc:
        with (
            tc.tile_pool(name="inp", bufs=1) as inp_pool,
            tc.tile_pool(name="ps", bufs=4, space="PSUM") as psum_pool,
            tc.tile_pool(name="outp", bufs=10) as out_pool,
        ):
            # Input pieces, in streaming order (= DRAM column order of `w`):
            #   tw0: [a0 cols 0:128 | b0 cols 0:1024]      -> first m-tile
            #   tw1: [a0 cols 128:1024 | b0 cols 1024:2048]
            #   tw2: [a0 cols 1024:2048]
            #   tw3: [b1 cols 0:2048]
            #   tw4: [a1 cols 0:2048]
            tw0 = inp_pool.tile([128, 1152], bf16, tag="tw0")
            tw1 = inp_pool.tile([128, 1920], bf16, tag="tw1")
            tw2 = inp_pool.tile([128, 1024], bf16, tag="tw2")
            tw3 = inp_pool.tile([128, 2048], bf16, tag="tw3")
            tw4 = inp_pool.tile([128, 2048], bf16, tag="tw4")
            pieces = [tw0, tw1, tw2, tw3, tw4]
            c0 = 0
            for tw in pieces:
                ncols = tw.shape[-1]
                nc.sync.dma_start(out=tw[:, :], in_=w[:, c0 : c0 + ncols])
                c0 += ncols

            # warm-up matmuls: keep the PE active (HAM clock ramp) while the
            # first input piece lands
            wps = psum_pool.tile([128, UNIT], f32, tag="ps")
            for _ in range(WARMUP_MM):
                nc.tensor.matmul(
                    wps[:, 0:512], wz[:, 0:128], wz[:, 0:512], start=True, stop=True
                )

            def lhs_ap(p, t, rows):
                if p == 1:
                    return tw4[rows, 128 * t : 128 * (t + 1)]
                if t == 0:
                    return tw0[rows, 0:128]
                if t < 8:
                    return tw1[rows, 128 * (t - 1) : 128 * t]
                return tw2[rows, 128 * (t - 8) : 128 * (t - 7)]

            def rhs_ap(p, n0, rows):
                if p == 1:
                    return tw3[rows, n0 : n0 + 512]
                if n0 < 1024:
                    return tw0[rows, 128 + n0 : 128 + n0 + 512]
                return tw1[rows, n0 - 128 : n0 + 384]  # 896 + (n0 - 1024)

            dve_t = act_t = 0.0

            def drain(o, hs, ps_ap, dve_ns, act_ns, force=None):
                nonlocal dve_t, act_t
                use_dve = dve_t + dve_ns <= act_t + act_ns
                if force is not None:
                    use_dve = force == "dve"
                if use_dve:
                    nc.vector.tensor_scalar_mul(o[:, hs], ps_ap, alpha)
                    dve_t += dve_ns
                else:
                    nc.scalar.activation(
                        o[:, hs],
                        ps_ap,
                        mybir.ActivationFunctionType.Copy,
                        scale=alpha,
                    )
                    act_t += act_ns

            for p in range(BPC // 2):       # batch pair
                # For pair 0, emit the first three m-tiles' h=0 units before
                # their h=1 units: the high b columns (in tw1) land a bit
                # after the low ones, and in-order emission would stall the
                # drain engines on t0's h=1 while t1/t2's h=0 data is ready.
                if p == 0:
                    order = [(0, 0), (1, 0), (2, 0), (0, 1), (1, 1), (2, 1)]
                    order += [(t, h) for t in range(3, MT) for h in range(NHALF)]
                else:
                    order = [(t, h) for t in range(MT) for h in range(NHALF)]
                otiles, hdone = {}, {}
                for t, h in order:
                    lhs0 = lhs_ap(p, t, slice(0, 64))
                    lhs1 = lhs_ap(p, t, slice(64, 128))
                    if t not in otiles:
                        o0 = out_pool.tile([128, N], i8, tag="o", name=f"o0_{p}_{t}")
                        o1 = out_pool.tile([128, N], i8, tag="o", name=f"o1_{p}_{t}")
                        otiles[t] = (o0, o1)
                        hdone[t] = 0
                    o0, o1 = otiles[t]
                    last = p == BPC // 2 - 1 and t == MT - 1
                    ps0 = psum_pool.tile([128, UNIT], f32, tag="ps", name=f"ps0_{p}_{t}_{h}")
                    ps1 = psum_pool.tile([128, UNIT], f32, tag="ps", name=f"ps1_{p}_{t}_{h}")
                    for j in range(2):  # 512-col matmul within unit
                        n0 = UNIT * h + 512 * j
                        c = slice(512 * j, 512 * (j + 1))
                        nc.tensor.matmul(
                            ps0[:, c],
                            lhs0,
                            rhs_ap(p, n0, slice(0, 64)),
                            start=True,
                            stop=True,
                        )
                        nc.tensor.matmul(
                            ps1[:, c],
                            lhs1,
                            rhs_ap(p, n0, slice(64, 128)),
                            start=True,
                            stop=True,
                        )
                    hs = slice(UNIT * h, UNIT * (h + 1))
                    if not last:
                        drain(o0, hs, ps0[:, :], DVE_NS, ACT_NS)
                        drain(o1, hs, ps1[:, :], DVE_NS, ACT_NS)
                    else:
                        # final m-tile: per-half 128KB output DMAs fired as
                        # soon as each half is drained, spread over sync /
                        # gpsimd / scalar trigger queues; the h=1 drains are
                        # pinned (o0->DVE with the sync trigger, o1->ACT with
                        # the scalar trigger) so the very last DMA trigger
                        # starts the moment its own engine finishes and waits
                        # on nothing else.
                        force0 = force1 = None
                        if h == 1:
                            force0, force1 = "dve", "act"
                        drain(o0, hs, ps0[:, :], DVE_NS, ACT_NS, force=force0)
                        nc.sync.dma_start(
                            out=out[2 * p, 128 * t : 128 * (t + 1), hs],
                            in_=o0[:, hs],
                        )
                        drain(o1, hs, ps1[:, :], DVE_NS, ACT_NS, force=force1)
                        if h == 0:
                            nc.gpsimd.dma_start(
                                out=out[2 * p + 1, 128 * t : 128 * (t + 1), hs],
                                in_=o1[:, hs],
                            )
                        else:
                            nc.scalar.dma_start(
                                out=out[2 * p + 1, 128 * t : 128 * (t + 1), hs],
                                in_=o1[:, hs],
                            )
                    hdone[t] += 1
                    if hdone[t] < NHALF or last:
                        continue
                    # full-tile output DMAs on the sync HWDGE ring (a
                    # dedicated queue that can afford to block on the
                    # cross-engine drain-completion waits)
                    nc.sync.dma_start(
                        out=out[2 * p, 128 * t : 128 * (t + 1), :], in_=o0[:, :]
                    )
                    nc.sync.dma_start(
                        out=out[2 * p + 1, 128 * t : 128 * (t + 1), :], in_=o1[:, :]
                    )
    nc.compile()
    return nc


def prep_in_maps(a: np.ndarray, b: np.ndarray):
    """Per-core packed input tensors: [K-pair, seq] bf16 pieces concatenated
    in on-device streaming order (see _build docstring)."""
    aT = np.ascontiguousarray(a.transpose(0, 2, 1)).astype(ml_dtypes.bfloat16)
    bT = np.ascontiguousarray(b.transpose(0, 2, 1)).astype(ml_dtypes.bfloat16)
    aT = aT.reshape(NCORES, BPC // 2, 128, M)
    bT = bT.reshape(NCORES, BPC // 2, 128, N)
    W = np.concatenate(
        [
            aT[:, 0, :, 0:128],
            bT[:, 0, :, 0:1024],
            aT[:, 0, :, 128:1024],
            bT[:, 0, :, 1024:2048],
            aT[:, 0, :, 1024:2048],
            bT[:, 1],
            aT[:, 1],
        ],
        axis=2,
    )
    assert W.shape == (NCORES, 128, WCOLS)
    return [{"w": np.ascontiguousarray(W[c])} for c in range(NCORES)]


def kernel(a: np.ndarray, b: np.ndarray, alpha) -> np.ndarray:
    from concourse.bass_utils import run_bass_kernel_spmd

    a = np.asarray(a)
    b = np.asarray(b)
    alpha_f = float(np.asarray(alpha))

    key = alpha_f
    if key not in _CACHE:
        _CACHE[key] = _build(alpha_f)
    nc = _CACHE[key]

    in_maps = prep_in_maps(a, b)
    try:
        res = run_bass_kernel_spmd(nc, in_maps, core_ids=list(range(NCORES)))
    except Exception:
        # one retry in case a previous process left a device in a bad state
        res = run_bass_kernel_spmd(nc, in_maps, core_ids=list(range(NCORES)))
    outs = [res.results[c]["out"] for c in range(NCORES)]
    return np.concatenate(outs, axis=0).astype(np.int8)


# revision 10
# speedup vs baseline: 1.0205x; 1.0205x over previous
"""Bass/TRN2 kernel for nn_BMM_S8T_S8N_S8T:
    out[b,m,n] = sat_i8(round(alpha * sum_k a[b,m,k] * b[b,n,k]))
with a: (32, 2048, 64) int8, b: (32, 2048, 64) int8, alpha: f32 scalar.

Sharding: batch dim 32 -> 8 cores x 4 batches (pure data parallel, no
cross-core communication).

Per-core design notes:
  - int8 matmul is not supported by the PE; bf16 x bf16 -> f32 PSUM is exact
    for int8 operands (products < 2^16, sums of 64 < 2^24), so inputs are
    converted to bf16 (and pre-transposed to [K, seq] layout) on host.
  - The 4 local batches are stacked in pairs along SBUF partitions:
    partitions 0-63 hold batch 2p's K=64, partitions 64-127 batch 2p+1's.
    Row-tiled matmuls (row groups 0 and 2) run the two batches' K=64
    contractions on the 128x128 PE array.
  - Requant drain (PSUM f32 -> SBUF int8, scale + round-half-even + saturate)
    is the hard floor: only VectorE (0.96 GHz, 1 elem/lane/cyc from f32 PSUM)
    and ScalarE (1.2 GHz, 1 elem/lane/cyc) can read PSUM. Both engines drain
    in parallel on different PSUM banks, in [128, 1024] (2-bank) units so
    fills overlap drains within the 8 banks. A single tensor_scalar_mul /
    activation(Copy, scale) does the whole requant bit-exactly.
  - The 16 DMA engines are per-descriptor bound and serve descriptors in
    global trigger order, so all per-core input data is host-repacked into
    ONE dram tensor `w` laid out in exact streaming order and loaded as 5
    sequential pieces on the sync HWDGE; the first piece alone carries
    everything the first m-tile needs. Output DMAs ride the sync HWDGE too
    (a dedicated queue whose cross-engine waits stall nothing); the final
    m-tile's four [128,1024] halves are DMA'd individually from sync /
    gpsimd / scalar with the last two drains pinned to a known engine, so
    the exit tail is a single 128KB transfer.
"""

import numpy as np
import ml_dtypes

B, M, N, K = 32, 2048, 2048, 64
NCORES = 8
BPC = B // NCORES          # batches per core (4)
MT = M // 128              # m-tiles per batch (16)
NHALF = 2                  # two 1024-col drain units per m-tile row block
UNIT = N // NHALF          # 1024 columns per drain unit
WCOLS = 4 * M              # packed input tensor: 8192 bf16 columns

_CACHE = {}


def _build(alpha: float):
    import concourse.bacc as bacc
    import concourse.mybir as mybir
    from concourse.tile import TileContext

    bf16 = mybir.dt.bfloat16
    f32 = mybir.dt.float32
    i8 = mybir.dt.int8

    nc = bacc.Bacc("TRN2")
    w = nc.dram_tensor("w", (128, WCOLS), bf16, kind="ExternalInput")
    out = nc.dram_tensor("out", (BPC, M, N), i8, kind="ExternalOutput")

    # engine load balancing between the two drain engines (ns per [128,1024]
    # unit, hardware-measured)
    DVE_NS, ACT_NS = 1146.0, 1035.0
    WARMUP_MM = 4  # filler matmuls while the first input piece streams in

    # scratch operand for warm-up matmuls: deliberately never written (the
    # values don't matter and the scratch PSUM bank is never read); a raw
    # (non-pool) tensor so Tile's release tracking doesn't object.
    wz = nc.alloc_sbuf_tensor("wz", [128, 512], bf16)

    with TileContext(nc) as tc:
        with (
            tc.tile_pool(name="inp", bufs=1) as inp_pool,
            tc.tile_pool(name="ps", bufs=4, space="PSUM") as psum_pool,
            tc.tile_pool(name="outp", bufs=10) as out_pool,
        ):
            # Input pieces, in streaming order (= DRAM column order of `w`).
            # The first m-tile's lhs (a0 cols 0:128) loads on the scalar
            # HWDGE in parallel with the first b piece's trigger generation
            # on the sync HWDGE; everything else follows on sync in order of
            # first use — the 16 DMA engines serve descriptors in global
            # trigger order, so this order IS the completion order.
            #   tw0: [a0 cols 0:128 | b0 cols 0:1024]      -> first m-tile
            #   tw1: [a0 cols 128:1024 | b0 cols 1024:2048]
            #   tw2: [a0 cols 1024:2048]
            #   tw3: [b1 cols 0:2048]
            #   tw4: [a1 cols 0:2048]
            tw0 = inp_pool.tile([128, 1152], bf16, tag="tw0")
            tw1 = inp_pool.tile([128, 1920], bf16, tag="tw1")
            tw2 = inp_pool.tile([128, 1024], bf16, tag="tw2")
            tw3 = inp_pool.tile([128, 2048], bf16, tag="tw3")
            tw4 = inp_pool.tile([128, 2048], bf16, tag="tw4")
            nc.scalar.dma_start(out=tw0[:, 0:128], in_=w[:, 0:128])
            nc.sync.dma_start(out=tw0[:, 128:640], in_=w[:, 128:640])
            nc.sync.dma_start(out=tw0[:, 640:1152], in_=w[:, 640:1152])
            nc.sync.dma_start(out=tw1[:, 0:896], in_=w[:, 1152:2048])
            nc.sync.dma_start(out=tw1[:, 896:1920], in_=w[:, 2048:3072])
            nc.sync.dma_start(out=tw2[:, :], in_=w[:, 3072:4096])
            nc.sync.dma_start(out=tw3[:, :], in_=w[:, 4096:6144])
            nc.sync.dma_start(out=tw4[:, :], in_=w[:, 6144:8192])

            # warm-up matmuls: keep the PE active (HAM clock ramp) while the
            # first input piece lands
            wps = psum_pool.tile([128, UNIT], f32, tag="ps")
            for _ in range(WARMUP_MM):
                nc.tensor.matmul(
                    wps[:, 0:512], wz[:, 0:128], wz[:, 0:512], start=True, stop=True
                )

            def lhs_ap(p, t, rows):
                if p == 1:
                    return tw4[rows, 128 * t : 128 * (t + 1)]
                if t == 0:
                    return tw0[rows, 0:128]
                if t < 8:
                    return tw1[rows, 128 * (t - 1) : 128 * t]
                return tw2[rows, 128 * (t - 8) : 128 * (t - 7)]

            def rhs_ap(p, n0, rows):
                if p == 1:
                    return tw3[rows, n0 : n0 + 512]
                if n0 < 1024:
                    return tw0[rows, 128 + n0 : 128 + n0 + 512]
                return tw1[rows, n0 - 128 : n0 + 384]  # 896 + (n0 - 1024)

            dve_t = act_t = 0.0

            def drain(o, hs, ps_ap, dve_ns, act_ns, force=None):
                nonlocal dve_t, act_t
                use_dve = dve_t + dve_ns <= act_t + act_ns
                if force is not None:
                    use_dve = force == "dve"
                if use_dve:
                    nc.vector.tensor_scalar_mul(o[:, hs], ps_ap, alpha)
                    dve_t += dve_ns
                else:
                    nc.scalar.activation(
                        o[:, hs],
                        ps_ap,
                        mybir.ActivationFunctionType.Copy,
                        scale=alpha,
                    )
                    act_t += act_ns

            for p in range(BPC // 2):       # batch pair
                # For pair 0, emit the first three m-tiles' h=0 units before
                # their h=1 units: the high b columns (in tw1) land a bit
                # after the low ones, and in-order emission would stall the
                # drain engines on t0's h=1 while t1/t2's h=0 data is ready.
                if p == 0:
                    order = [(0, 0), (1, 0), (2, 0), (0, 1), (1, 1), (2, 1)]
                    order += [(t, h) for t in range(3, MT) for h in range(NHALF)]
                else:
                    order = [(t, h) for t in range(MT) for h in range(NHALF)]
                otiles, hdone = {}, {}
                for t, h in order:
                    lhs0 = lhs_ap(p, t, slice(0, 64))
                    lhs1 = lhs_ap(p, t, slice(64, 128))
                    if t not in otiles:
                        o0 = out_pool.tile([128, N], i8, tag="o", name=f"o0_{p}_{t}")
                        o1 = out_pool.tile([128, N], i8, tag="o", name=f"o1_{p}_{t}")
                        otiles[t] = (o0, o1)
                        hdone[t] = 0
                    o0, o1 = otiles[t]
                    last = p == BPC // 2 - 1 and t == MT - 1
                    ps0 = psum_pool.tile([128, UNIT], f32, tag="ps", name=f"ps0_{p}_{t}_{h}")
                    ps1 = psum_pool.tile([128, UNIT], f32, tag="ps", name=f"ps1_{p}_{t}_{h}")
                    for j in range(2):  # 512-col matmul within unit
                        n0 = UNIT * h + 512 * j
                        c = slice(512 * j, 512 * (j + 1))
                        nc.tensor.matmul(
                            ps0[:, c],
                            lhs0,
                            rhs_ap(p, n0, slice(0, 64)),
                            start=True,
                            stop=True,
                        )
                        nc.tensor.matmul(
                            ps1[:, c],
                            lhs1,
                            rhs_ap(p, n0, slice(64, 128)),
                            start=True,
                            stop=True,
                        )
                    hs = slice(UNIT * h, UNIT * (h + 1))
                    if not last:
                        drain(o0, hs, ps0[:, :], DVE_NS, ACT_NS)
                        drain(o1, hs, ps1[:, :], DVE_NS, ACT_NS)
                    else:
                        # final m-tile: per-half 128KB output DMAs fired as
                        # soon as each half is drained, spread over sync /
                        # gpsimd / scalar trigger queues; the h=1 drains are
                        # pinned (o0->DVE with the sync trigger, o1->ACT with
                        # the scalar trigger) so the very last DMA trigger
                        # starts the moment its own engine finishes and waits
                        # on nothing else.
                        force0 = force1 = None
                        if h == 1:
                            force0, force1 = "dve", "act"
                        drain(o0, hs, ps0[:, :], DVE_NS, ACT_NS, force=force0)
                        nc.sync.dma_start(
                            out=out[2 * p, 128 * t : 128 * (t + 1), hs],
                            in_=o0[:, hs],
                        )
                        drain(o1, hs, ps1[:, :], DVE_NS, ACT_NS, force=force1)
                        if h == 0:
                            nc.gpsimd.dma_start(
                                out=out[2 * p + 1, 128 * t : 128 * (t + 1), hs],
                                in_=o1[:, hs],
                            )
                        else:
                            nc.scalar.dma_start(
                                out=out[2 * p + 1, 128 * t : 128 * (t + 1), hs],
                                in_=o1[:, hs],
                            )
                    hdone[t] += 1
                    if hdone[t] < NHALF or last:
                        continue
                    # full-tile output DMAs on the sync HWDGE ring (a
                    # dedicated queue that can afford to block on the
                    # cross-engine drain-completion waits)
                    nc.sync.dma_start(
                        out=out[2 * p, 128 * t : 128 * (t + 1), :], in_=o0[:, :]
                    )
                    nc.sync.dma_start(
                        out=out[2 * p + 1, 128 * t : 128 * (t + 1), :], in_=o1[:, :]
                    )
    nc.compile()
    return nc


def prep_in_maps(a: np.ndarray, b: np.ndarray):
    """Per-core packed input tensors: [K-pair, seq] bf16 pieces concatenated
    in on-device streaming order (see _build docstring)."""
    aT = np.ascontiguousarray(a.transpose(0, 2, 1)).astype(ml_dtypes.bfloat16)
    bT = np.ascontiguousarray(b.transpose(0, 2, 1)).astype(ml_dtypes.bfloat16)
    aT = aT.reshape(NCORES, BPC // 2, 128, M)
    bT = bT.reshape(NCORES, BPC // 2, 128, N)
    W = np.concatenate(
        [
            aT[:, 0, :, 0:128],
            bT[:, 0, :, 0:1024],
            aT[:, 0, :, 128:1024],
            bT[:, 0, :, 1024:2048],
            aT[:, 0, :, 1024:2048],
            bT[:, 1],
            aT[:, 1],
        ],
        axis=2,
    )
    assert W.shape == (NCORES, 128, WCOLS)
    return [{"w": np.ascontiguousarray(W[c])} for c in range(NCORES)]


def kernel(a: np.ndarray, b: np.ndarray, alpha) -> np.ndarray:
    from concourse.bass_utils import run_bass_kernel_spmd

    a = np.asarray(a)
    b = np.asarray(b)
    alpha_f = float(np.asarray(alpha))

    key = alpha_f
    if key not in _CACHE:
        _CACHE[key] = _build(alpha_f)
    nc = _CACHE[key]

    in_maps = prep_in_maps(a, b)
    try:
        res = run_bass_kernel_spmd(nc, in_maps, core_ids=list(range(NCORES)))
    except Exception:
        # one retry in case a previous process left a device in a bad state
        res = run_bass_kernel_spmd(nc, in_maps, core_ids=list(range(NCORES)))
    outs = [res.results[c]["out"] for c in range(NCORES)]
    return np.concatenate(outs, axis=0).astype(np.int8)


# revision 11
# speedup vs baseline: 1.0209x; 1.0003x over previous
"""Bass/TRN2 kernel for nn_BMM_S8T_S8N_S8T:
    out[b,m,n] = sat_i8(round(alpha * sum_k a[b,m,k] * b[b,n,k]))
with a: (32, 2048, 64) int8, b: (32, 2048, 64) int8, alpha: f32 scalar.

Sharding: batch dim 32 -> 8 cores x 4 batches (pure data parallel, no
cross-core communication).

Per-core design notes:
  - int8 matmul is not supported by the PE; bf16 x bf16 -> f32 PSUM is exact
    for int8 operands (products < 2^16, sums of 64 < 2^24), so inputs are
    converted to bf16 (and pre-transposed to [K, seq] layout) on host.
  - The 4 local batches are stacked in pairs along SBUF partitions:
    partitions 0-63 hold batch 2p's K=64, partitions 64-127 batch 2p+1's.
    Row-tiled matmuls (row groups 0 and 2) run the two batches' K=64
    contractions on the 128x128 PE array.
  - Requant drain (PSUM f32 -> SBUF int8, scale + round-half-even + saturate)
    is the hard floor: only VectorE (0.96 GHz, 1 elem/lane/cyc from f32 PSUM)
    and ScalarE (1.2 GHz, 1 elem/lane/cyc) can read PSUM. Both engines drain
    in parallel on different PSUM banks, in [128, 1024] (2-bank) units so
    fills overlap drains within the 8 banks. A single tensor_scalar_mul /
    activation(Copy, scale) does the whole requant bit-exactly.
  - The 16 DMA engines are per-descriptor bound and serve descriptors in
    global trigger order, so all per-core input data is host-repacked into
    ONE dram tensor `w` laid out in exact streaming order and loaded as 5
    sequential pieces on the sync HWDGE; the first piece alone carries
    everything the first m-tile needs. Output DMAs ride the sync HWDGE too
    (a dedicated queue whose cross-engine waits stall nothing); the final
    m-tile's four [128,1024] halves are DMA'd individually from sync /
    gpsimd / scalar with the last two drains pinned to a known engine, so
    the exit tail is a single 128KB transfer.
"""

import numpy as np
import ml_dtypes

B, M, N, K = 32, 2048, 2048, 64
NCORES = 8
BPC = B // NCORES          # batches per core (4)
MT = M // 128              # m-tiles per batch (16)
NHALF = 2                  # two 1024-col drain units per m-tile row block
UNIT = N // NHALF          # 1024 columns per drain unit
WCOLS = 4 * M              # packed input tensor: 8192 bf16 columns

_CACHE = {}


def _build(alpha: float):
    import concourse.bacc as bacc
    import concourse.mybir as mybir
    from concourse.tile import TileContext

    bf16 = mybir.dt.bfloat16
    f32 = mybir.dt.float32
    i8 = mybir.dt.int8

    nc = bacc.Bacc("TRN2")
    w = nc.dram_tensor("w", (128, WCOLS), bf16, kind="ExternalInput")
    out = nc.dram_tensor("out", (BPC, M, N), i8, kind="ExternalOutput")

    # engine load balancing between the two drain engines (ns per [128,1024]
    # unit, hardware-measured)
    DVE_NS, ACT_NS = 1146.0, 1035.0
    WARMUP_MM = 4  # filler matmuls while the first input pieces stream in

    # scratch operand for warm-up matmuls: deliberately never written (the
    # values don't matter and the scratch PSUM bank is never read); a raw
    # (non-pool) tensor so Tile's release tracking doesn't object.
    wz = nc.alloc_sbuf_tensor("wz", [128, 512], bf16)

    with TileContext(nc) as tc:
        with (
            tc.tile_pool(name="inp", bufs=1) as inp_pool,
            tc.tile_pool(name="ps", bufs=4, space="PSUM") as psum_pool,
            tc.tile_pool(name="outp", bufs=10) as out_pool,
        ):
            # Input pieces, in streaming order (= DRAM column order of `w`).
            # The first m-tile's lhs (a0 cols 0:128) loads on the scalar
            # HWDGE in parallel with the first b piece's trigger generation
            # on the sync HWDGE; everything else follows on sync in order of
            # first use — the 16 DMA engines serve descriptors in global
            # trigger order, so this order IS the completion order.
            #   tw0: [a0 cols 0:128 | b0 cols 0:1024]      -> first m-tile
            #   tw1: [a0 cols 128:1024 | b0 cols 1024:2048]
            #   tw2: [a0 cols 1024:2048]
            #   tw3: [b1 cols 0:2048]
            #   tw4: [a1 cols 0:2048]
            tw0 = inp_pool.tile([128, 1152], bf16, tag="tw0")
            tw1 = inp_pool.tile([128, 1920], bf16, tag="tw1")
            tw2 = inp_pool.tile([128, 1024], bf16, tag="tw2")
            tw3 = inp_pool.tile([128, 2048], bf16, tag="tw3")
            tw4 = inp_pool.tile([128, 2048], bf16, tag="tw4")
            nc.scalar.dma_start(out=tw0[:, 0:128], in_=w[:, 0:128])
            nc.sync.dma_start(out=tw0[:, 128:640], in_=w[:, 128:640])
            nc.sync.dma_start(out=tw0[:, 640:1152], in_=w[:, 640:1152])
            nc.sync.dma_start(out=tw1[:, 0:896], in_=w[:, 1152:2048])
            nc.sync.dma_start(out=tw1[:, 896:1920], in_=w[:, 2048:3072])
            nc.sync.dma_start(out=tw2[:, :], in_=w[:, 3072:4096])
            nc.sync.dma_start(out=tw3[:, :], in_=w[:, 4096:6144])
            nc.sync.dma_start(out=tw4[:, :], in_=w[:, 6144:8192])

            # warm-up matmuls: keep the PE active (HAM clock ramp) while the
            # first input piece lands
            wps = psum_pool.tile([128, UNIT], f32, tag="ps")
            for _ in range(WARMUP_MM):
                nc.tensor.matmul(
                    wps[:, 0:512], wz[:, 0:128], wz[:, 0:512], start=True, stop=True
                )

            def lhs_ap(p, t, rows):
                if p == 1:
                    return tw4[rows, 128 * t : 128 * (t + 1)]
                if t == 0:
                    return tw0[rows, 0:128]
                if t < 8:
                    return tw1[rows, 128 * (t - 1) : 128 * t]
                return tw2[rows, 128 * (t - 8) : 128 * (t - 7)]

            def rhs_ap(p, n0, rows):
                if p == 1:
                    return tw3[rows, n0 : n0 + 512]
                if n0 < 1024:
                    return tw0[rows, 128 + n0 : 128 + n0 + 512]
                return tw1[rows, n0 - 128 : n0 + 384]  # 896 + (n0 - 1024)

            dve_t = act_t = 0.0

            def drain(o, hs, ps_ap, dve_ns, act_ns, force=None):
                nonlocal dve_t, act_t
                use_dve = dve_t + dve_ns <= act_t + act_ns
                if force is not None:
                    use_dve = force == "dve"
                if use_dve:
                    nc.vector.tensor_scalar_mul(o[:, hs], ps_ap, alpha)
                    dve_t += dve_ns
                else:
                    nc.scalar.activation(
                        o[:, hs],
                        ps_ap,
                        mybir.ActivationFunctionType.Copy,
                        scale=alpha,
                    )
                    act_t += act_ns

            for p in range(BPC // 2):       # batch pair
                # For pair 0, emit the first three m-tiles' h=0 units before
                # their h=1 units: the high b columns (in tw1) land a bit
                # after the low ones, and in-order emission would stall the
                # drain engines on t0's h=1 while t1/t2's h=0 data is ready.
                if p == 0:
                    order = [(0, 0), (1, 0), (2, 0), (0, 1), (1, 1), (2, 1)]
                    order += [(t, h) for t in range(3, MT) for h in range(NHALF)]
                else:
                    order = [(t, h) for t in range(MT) for h in range(NHALF)]
                otiles, hdone = {}, {}
                for t, h in order:
                    lhs0 = lhs_ap(p, t, slice(0, 64))
                    lhs1 = lhs_ap(p, t, slice(64, 128))
                    if t not in otiles:
                        o0 = out_pool.tile([128, N], i8, tag="o", name=f"o0_{p}_{t}")
                        o1 = out_pool.tile([128, N], i8, tag="o", name=f"o1_{p}_{t}")
                        otiles[t] = (o0, o1)
                        hdone[t] = 0
                    o0, o1 = otiles[t]
                    last = p == BPC // 2 - 1 and t == MT - 1
                    ps0 = psum_pool.tile([128, UNIT], f32, tag="ps", name=f"ps0_{p}_{t}_{h}")
                    ps1 = psum_pool.tile([128, UNIT], f32, tag="ps", name=f"ps1_{p}_{t}_{h}")
                    for j in range(2):  # 512-col matmul within unit
                        n0 = UNIT * h + 512 * j
                        c = slice(512 * j, 512 * (j + 1))
                        nc.tensor.matmul(
                            ps0[:, c],
                            lhs0,
                            rhs_ap(p, n0, slice(0, 64)),
                            start=True,
                            stop=True,
                        )
                        nc.tensor.matmul(
                            ps1[:, c],
                            lhs1,
                            rhs_ap(p, n0, slice(64, 128)),
                            start=True,
                            stop=True,
                        )
                    hs = slice(UNIT * h, UNIT * (h + 1))
                    if not last:
                        drain(o0, hs, ps0[:, :], DVE_NS, ACT_NS)
                        drain(o1, hs, ps1[:, :], DVE_NS, ACT_NS)
                    else:
                        # final m-tile: per-half 128KB output DMAs fired as
                        # soon as each half is drained, spread over sync /
                        # gpsimd / scalar trigger queues; the h=1 drains are
                        # pinned (o0->DVE with the sync trigger, o1->ACT with
                        # the scalar trigger) so the very last DMA trigger
                        # starts the moment its own engine finishes and waits
                        # on nothing else.
                        force0 = force1 = None
                        if h == 1:
                            force0, force1 = "dve", "act"
                        drain(o0, hs, ps0[:, :], DVE_NS, ACT_NS, force=force0)
                        nc.sync.dma_start(
                            out=out[2 * p, 128 * t : 128 * (t + 1), hs],
                            in_=o0[:, hs],
                        )
                        drain(o1, hs, ps1[:, :], DVE_NS, ACT_NS, force=force1)
                        if h == 0:
                            nc.gpsimd.dma_start(
                                out=out[2 * p + 1, 128 * t : 128 * (t + 1), hs],
                                in_=o1[:, hs],
                            )
                        else:
                            nc.scalar.dma_start(
                                out=out[2 * p + 1, 128 * t : 128 * (t + 1), hs],
                                in_=o1[:, hs],
                            )
                    hdone[t] += 1
                    if hdone[t] < NHALF or last:
                        continue
                    # full-tile output DMAs on the sync HWDGE ring (a
                    # dedicated queue that can afford to block on the
                    # cross-engine drain-completion waits)
                    nc.sync.dma_start(
                        out=out[2 * p, 128 * t : 128 * (t + 1), :], in_=o0[:, :]
                    )
                    nc.sync.dma_start(
                        out=out[2 * p + 1, 128 * t : 128 * (t + 1), :], in_=o1[:, :]
                    )
    nc.compile()
    return nc


def prep_in_maps(a: np.ndarray, b: np.ndarray):
    """Per-core packed input tensors: [K-pair, seq] bf16 pieces concatenated
    in on-device streaming order (see _build docstring)."""
    aT = np.ascontiguousarray(a.transpose(0, 2, 1)).astype(ml_dtypes.bfloat16)
    bT = np.ascontiguousarray(b.transpose(0, 2, 1)).astype(ml_dtypes.bfloat16)
    aT = aT.reshape(NCORES, BPC // 2, 128, M)
    bT = bT.reshape(NCORES, BPC // 2, 128, N)
    W = np.concatenate(
        [
            aT[:, 0, :, 0:128],
            bT[:, 0, :, 0:1024],
            aT[:, 0, :, 128:1024],
            bT[:, 0, :, 1024:2048],
            aT[:, 0, :, 1024:2048],
            bT[:, 1],
            aT[:, 1],
        ],
        axis=2,
    )
    assert W.shape == (NCORES, 128, WCOLS)
    return [{"w": np.ascontiguousarray(W[c])} for c in range(NCORES)]


def kernel(a: np.ndarray, b: np.ndarray, alpha) -> np.ndarray:
    from concourse.bass_utils import run_bass_kernel_spmd

    a = np.asarray(a)
    b = np.asarray(b)
    alpha_f = float(np.asarray(alpha))

    key = alpha_f
    if key not in _CACHE:
        _CACHE[key] = _build(alpha_f)
    nc = _CACHE[key]

    in_maps = prep_in_maps(a, b)
    try:
        res = run_bass_kernel_spmd(nc, in_maps, core_ids=list(range(NCORES)))
    except Exception:
        # one retry in case a previous process left a device in a bad state
        res = run_bass_kernel_spmd(nc, in_maps, core_ids=list(range(NCORES)))
    outs = [res.results[c]["out"] for c in range(NCORES)]
    return np.concatenate(outs, axis=0).astype(np.int8)
